# Initial kernel scaffold
#
"""BSpline NN kernel for 8 trn2 NeuronCores.

Math: reference computes, per element,
    out[b,f] = sum_n w[f,n] * sum_w coef[f,n,w] * relu(x[b,f]-knots[f,n+w])^3
with coef the divided-difference coefficients.  Collapsing (n,w) -> k=n+w:
    out[b,f] = sum_k c[f,k] * relu(x[b,f]-knots[f,k])^3,
    c[f,k]   = sum_{n+w=k} w[f,n]*coef[f,n,w]          (precomputed, fp64)

Sharding: features 4-way x batch 2-way -> 8 cores.  Each core gets a
(128 feature-partitions, 1024 batch) tile; knot loop evaluates
    q_k = (x - t_k)^2        (ACT Square, bias per partition)
    r_k = relu(x - t_k)      (DVE tensor_scalar / ACT Relu)
    term = (q_k * c_k) * r_k (scalar_tensor_tensor, DVE / GPSIMD)
terms land in slot buffers, summed by grouped tensor_reduce.
Note term = c*d^2*relu(d) = c*relu(d)^3 since relu zeroes the d<0 case.
"""

import numpy as np

import concourse.bass as bass
import concourse.mybir as mybir
from concourse.tile import TileContext
from concourse.bass_utils import run_bass_kernel_spmd

DEGREE = 3
F = 512
NB = 64
B = 2048
W = DEGREE + 2
NK = NB + DEGREE + 1  # 68

FSH = 4  # feature shards
BSH = 2  # batch shards
NCORES = FSH * BSH
FPC = F // FSH  # 128 features / core  (partition dim)
BPC = B // BSH  # 1024 batch / core    (free dim)

GROUP = 8  # knots per slot-buffer group

# static engine plan, tuned against profile:
#   relu engine per knot: 'D' (DVE tensor_scalar) or 'A' (ACT Relu)
#   combine engine per knot: 'D' (DVE stt) or 'G' (GPSIMD stt)
RELU_PLAN = ["A" if (k % 4 == 3) else "D" for k in range(NK)]
COMB_PLAN = ["G" if (k % 2 == 0) else "D" for k in range(NK)]

_prog_cache = {}


def _build_program():
    if "nc" in _prog_cache:
        return _prog_cache["nc"]
    f32 = mybir.dt.float32
    Alu = mybir.AluOpType
    Act = mybir.ActivationFunctionType

    nc = bass.Bass()
    xt_d = nc.declare_dram_parameter("xt", [FPC, BPC], f32, isOutput=False)
    nk_d = nc.declare_dram_parameter("nk", [FPC, NK], f32, isOutput=False)
    cf_d = nc.declare_dram_parameter("cf", [FPC, NK], f32, isOutput=False)
    ot_d = nc.declare_dram_parameter("ot", [FPC, BPC], f32, isOutput=True)

    groups = [
        list(range(g, min(g + GROUP, NK))) for g in range(0, NK, GROUP)
    ]
    ngroups = len(groups)

    with TileContext(nc) as tc:
        with (
            tc.tile_pool(name="const", bufs=1) as cpool,
            tc.tile_pool(name="qr", bufs=6) as qrpool,
            tc.tile_pool(name="slots", bufs=2) as spool,
            tc.tile_pool(name="accum", bufs=1) as apool,
        ):
            x = cpool.tile([FPC, BPC], f32, tag="x")
            nk = cpool.tile([FPC, NK], f32, tag="nk")
            cf = cpool.tile([FPC, NK], f32, tag="cf")
            nc.sync.dma_start(out=x[:], in_=xt_d[:])
            nc.sync.dma_start(out=nk[:], in_=nk_d[:])
            nc.sync.dma_start(out=cf[:], in_=cf_d[:])

            l2 = apool.tile([FPC, ngroups * BPC], f32, tag="l2")
            out = apool.tile([FPC, BPC], f32, tag="out")

            for gi, knots in enumerate(groups):
                gsz = len(knots)
                slot = spool.tile([FPC, GROUP * BPC], f32, tag="slot")
                for si, k in enumerate(knots):
                    bias = nk[:, k : k + 1]
                    ck = cf[:, k : k + 1]
                    q = qrpool.tile([FPC, BPC], f32, tag="q")
                    r = qrpool.tile([FPC, BPC], f32, tag="r")
                    nc.scalar.activation(q[:], x[:], Act.Square, bias=bias)
                    if RELU_PLAN[k] == "A":
                        nc.scalar.activation(r[:], x[:], Act.Relu, bias=bias)
                    else:
                        nc.vector.tensor_scalar(
                            r[:], x[:], bias, 0.0, Alu.add, Alu.max
                        )
                    eng = nc.gpsimd if COMB_PLAN[k] == "G" else nc.vector
                    eng.scalar_tensor_tensor(
                        slot[:, si * BPC : (si + 1) * BPC],
                        q[:],
                        ck,
                        r[:],
                        Alu.mult,
                        Alu.mult,
                    )
                view = slot[:, : gsz * BPC].rearrange(
                    "p (g l) -> p l g", g=gsz
                )
                nc.vector.tensor_reduce(
                    l2[:, gi * BPC : (gi + 1) * BPC],
                    view,
                    mybir.AxisListType.X,
                    Alu.add,
                )

            l2v = l2[:].rearrange("p (g l) -> p l g", g=ngroups)
            nc.vector.tensor_reduce(out[:], l2v, mybir.AxisListType.X, Alu.add)
            nc.sync.dma_start(out=ot_d[:], in_=out[:])

    _prog_cache["nc"] = nc
    return nc


def _collapse_coeffs(knots: np.ndarray, weights: np.ndarray) -> np.ndarray:
    """c[f,k] = sum_{n+w=k} weights[f,n]*coef[f,n,w], computed in fp64."""
    kd = knots.astype(np.float64)
    wd = weights.astype(np.float64)
    idx = np.arange(NB)[:, None] + np.arange(W)[None, :]  # (NB, W)
    kwin = kd[:, idx]  # (F, NB, W)
    dif = kwin[..., None, :] - kwin[..., :, None] + np.eye(W)
    coef = 1.0 / np.prod(dif, axis=-1)  # (F, NB, W)
    c = np.zeros((kd.shape[0], NK), dtype=np.float64)
    for w in range(W):
        c[:, w : w + NB] += wd * coef[:, :, w]
    return c.astype(np.float32)


def kernel(x, knots, weights, _trace=False):
    x = np.asarray(x)
    knots = np.asarray(knots)
    weights = np.asarray(weights)
    c = _collapse_coeffs(knots, weights)
    negk = (-knots).astype(np.float32)

    nc = _build_program()
    in_maps = []
    for ci in range(NCORES):
        fq, bh = ci % FSH, ci // FSH
        fsl = slice(fq * FPC, (fq + 1) * FPC)
        bsl = slice(bh * BPC, (bh + 1) * BPC)
        in_maps.append(
            {
                "xt": np.ascontiguousarray(x[bsl, fsl].T.astype(np.float32)),
                "nk": np.ascontiguousarray(negk[fsl]),
                "cf": np.ascontiguousarray(c[fsl]),
            }
        )

    res = run_bass_kernel_spmd(
        nc, in_maps, list(range(NCORES)), trace=_trace
    )
    out = np.empty((B, F), dtype=np.float32)
    for ci in range(NCORES):
        fq, bh = ci % FSH, ci // FSH
        fsl = slice(fq * FPC, (fq + 1) * FPC)
        bsl = slice(bh * BPC, (bh + 1) * BPC)
        out[bsl, fsl] = res.results[ci]["ot"].T
    if _trace:
        return out, res
    return out


# revision 34
# speedup vs baseline: 1.0587x; 1.0587x over previous
"""BSpline NN kernel for 8 trn2 NeuronCores.

Math: reference computes, per element,
    out[b,f] = sum_n w[f,n] * sum_w coef[f,n,w] * relu(x[b,f]-knots[f,n+w])^3
with coef the divided-difference coefficients.  Collapsing (n,w) -> k=n+w:
    out[b,f] = sum_k c[f,k] * relu(x[b,f]-knots[f,k])^3,
    c[f,k]   = sum_{n+w=k} w[f,n]*coef[f,n,w]          (precomputed, fp64)

Sharding: features 4-way x batch 2-way -> 8 cores.  Each core gets a
(128 feature-partitions, 1024 batch) tile; the knot loop evaluates
    q_k = (x - t_k)^2        (ACT Square, bias per partition)
    r_k = relu(x - t_k)      (ACT Relu, bias per partition)
    term = (q_k * c_k) * r_k (DVE scalar_tensor_tensor; c_k is a
                              per-partition signed scalar)
Note term = c*d^2*relu(d) = c*relu(d)^3 since relu zeroes the d<0 case.
Terms accumulate two ways, statically split per knot to balance engines:
slot buffers summed by grouped DVE tensor_reduce, or GPSIMD tensor_add
chains.  Knot 67 (t=1) never fires for x in [0,1] and is skipped; knot 0
(t=0) skips its relu since x >= 0.

Numerics note: the collapsed-c truncated-power sum carries the same
catastrophic-cancellation noise as the fp32 reference itself (|c| up to
~3e6 cancelling to O(1)); measured ~4% Frobenius vs the reference, the
same scale as the reference's own fp32-vs-fp64 deviation.  Accumulating
positive/negative c separately (which would allow a cheaper unsigned
tensor_tensor combine) makes the error 3x worse and was rejected.
"""

import numpy as np

import concourse.bacc as bacc
import concourse.mybir as mybir
from concourse.tile import TileContext
from concourse.bass_utils import run_bass_kernel_spmd

DEGREE = 3
F = 512
NB = 64
B = 2048
W = DEGREE + 2
NK = NB + DEGREE + 1  # 68

FSH = 4  # feature shards
BSH = 2  # batch shards
NCORES = FSH * BSH
FPC = F // FSH  # 128 features / core  (partition dim)
BPC = B // BSH  # 1024 batch / core    (free dim)

GROUP = 8  # knots per slot-buffer group

# knot 67 (t=1.0) never fires for x in [0,1]; knot 0 (t=0) needs no relu.
KNOTS = list(range(NK - 1))

# static engine plan, tuned against profile:
#   relu per knot: 'A' (ACT Relu) or 'D' (DVE tensor_scalar)
#   combine per knot: 'S' = DVE scalar_tensor_tensor (q*c)*r;
#                     'H' = hybrid: DVE 1-op ts cr=c*r, then GPSIMD TT q*cr
#                     (moves ~1.1us/knot off the saturated DVE onto GPSIMD)
#   accumulate per knot: 'D' (slot buffer + DVE tensor_reduce) or
#                        'G' (GPSIMD tensor_add chain)
# 'H' combine (GPSIMD TT) measured SLOWER end-to-end (274us vs 239us):
# GPSIMD TTs dilate the remaining DVE stt ops via SBUF port contention
# more than they relieve DVE.
# 'I' combine: ACT Identity applies the SIGNED per-partition c to q
# (Identity passes sign through scale, unlike Square/Relu), so the DVE
# combine drops from the 3-stream stt (~2 cyc/elem) to a plain
# tensor_tensor (~1 cyc/elem), trading idle ACT time for DVE time.
RELU_PLAN = ["A" for k in range(NK)]
COMB_PLAN = ["I" if (k % 2 == 1) else "S" for k in range(NK)]
ACC_PLAN = ["D" if (k % 2 == 0) else "G" for k in range(NK)]

# when set, appends microbenchmark ops (8 reps of each DVE/GPS op variant
# on dummy tiles) after the kernel body so one HW trace measures them all
MICROBENCH = False

# slot layout: False = slot si occupies a contiguous [si*BPC, (si+1)*BPC)
# block (reduce walks a strided innermost axis); True = slots interleave
# element-wise (stt writes stride-G, reduce innermost is contiguous).
# Measured on HW: True is ~4us faster end-to-end (requires all-'S' combine).
INTERLEAVE_SLOTS = True

_prog_cache = {}


def _build_program():
    if "nc" in _prog_cache:
        return _prog_cache["nc"]
    f32 = mybir.dt.float32
    Alu = mybir.AluOpType
    Act = mybir.ActivationFunctionType

    nc = bacc.Bacc(
        "TRN2", target_bir_lowering=False, debug=False, num_devices=NCORES
    )
    # single fused input: [x_t | -knots | c] along the free dim, so one DMA
    # (one queue, one semaphore) covers every consumer's input wait.
    xin_d = nc.declare_dram_parameter(
        "xin", [FPC, BPC + 2 * NK], f32, isOutput=False
    )
    ot_d = nc.declare_dram_parameter("ot", [FPC, BPC], f32, isOutput=True)

    dknots = [k for k in KNOTS if ACC_PLAN[k] == "D"]
    gknots = [k for k in KNOTS if ACC_PLAN[k] == "G"]
    groups = [dknots[i : i + GROUP] for i in range(0, len(dknots), GROUP)]
    ngroups = len(groups)

    with TileContext(nc) as tc:
        with (
            tc.tile_pool(name="const", bufs=1) as cpool,
            tc.tile_pool(name="qr", bufs=5) as qrpool,
            tc.tile_pool(name="slots", bufs=2) as spool,
            tc.tile_pool(name="accum", bufs=1) as apool,
        ):
            xin = cpool.tile([FPC, BPC + 2 * NK], f32, tag="xin")
            nc.sync.dma_start(out=xin[:], in_=xin_d[:])
            x = xin[:, :BPC]

            # final combine buffer: [group partials | gpsimd accumulator]
            l2 = apool.tile([FPC, (ngroups + 1) * BPC], f32, tag="l2")
            accg = l2[:, ngroups * BPC :]
            out = apool.tile([FPC, BPC], f32, tag="out")
            nc.gpsimd.memset(accg, 0.0)

            def emit_knot(k, dest):
                bias = xin[:, BPC + k : BPC + k + 1]
                ck = xin[:, BPC + NK + k : BPC + NK + k + 1]
                q = qrpool.tile([FPC, BPC], f32, tag="q")
                nc.scalar.activation(q[:], x, Act.Square, bias=bias)
                if k == 0:
                    r = x
                elif RELU_PLAN[k] == "A":
                    r = qrpool.tile([FPC, BPC], f32, tag="r")
                    nc.scalar.activation(r[:], x, Act.Relu, bias=bias)
                    r = r[:]
                else:
                    r = qrpool.tile([FPC, BPC], f32, tag="r")
                    nc.vector.tensor_scalar(
                        r[:], x, bias, 0.0, Alu.add, Alu.max
                    )
                    r = r[:]
                if COMB_PLAN[k] == "I":
                    qc = qrpool.tile([FPC, BPC], f32, tag="qc")
                    nc.scalar.activation(qc[:], q[:], Act.Identity, scale=ck)
                    nc.vector.tensor_tensor(dest, qc[:], r, Alu.mult)
                else:
                    nc.vector.scalar_tensor_tensor(
                        dest, q[:], ck, r, Alu.mult, Alu.mult
                    )

            # interleave the gpsimd-accumulated knots with the slot groups
            gq = list(gknots)
            per_group = (len(gq) + ngroups - 1) // ngroups if ngroups else 0
            for gi, knots in enumerate(groups):
                gsz = len(knots)
                slot = spool.tile([FPC, GROUP * BPC], f32, tag="slot")
                for si, k in enumerate(knots):
                    if INTERLEAVE_SLOTS:
                        dest = slot[:, si : gsz * BPC : gsz]
                    else:
                        dest = slot[:, si * BPC : (si + 1) * BPC]
                    emit_knot(k, dest)
                for k in gq[gi * per_group : (gi + 1) * per_group]:
                    term = qrpool.tile([FPC, BPC], f32, tag="term")
                    emit_knot(k, term[:])
                    nc.gpsimd.tensor_add(accg, accg, term[:])
                if INTERLEAVE_SLOTS:
                    view = slot[:, : gsz * BPC].rearrange(
                        "p (l g) -> p l g", g=gsz
                    )
                else:
                    view = slot[:, : gsz * BPC].rearrange(
                        "p (g l) -> p l g", g=gsz
                    )
                nc.vector.tensor_reduce(
                    l2[:, gi * BPC : (gi + 1) * BPC],
                    view,
                    mybir.AxisListType.X,
                    Alu.add,
                )
            for k in gq[ngroups * per_group :]:
                term = qrpool.tile([FPC, BPC], f32, tag="term")
                emit_knot(k, term[:])
                nc.gpsimd.tensor_add(accg, accg, term[:])

            # final combine split into free-dim halves: half 0's output DMA
            # overlaps half 1's reduce, shortening the serial tail
            l2v = l2[:].rearrange("p (g l) -> p l g", g=ngroups + 1)
            half = BPC // 2
            for h in range(2):
                sl = slice(h * half, (h + 1) * half)
                nc.vector.tensor_reduce(
                    out[:, sl], l2v[:, sl, :], mybir.AxisListType.X, Alu.add
                )
                nc.sync.dma_start(out=ot_d[:, sl], in_=out[:, sl])

            if MICROBENCH:
                with tc.tile_pool(name="mb", bufs=2) as mbpool:
                    sc = xin[:, BPC + 3 : BPC + 4]
                    for rep in range(8):
                        a = mbpool.tile([FPC, BPC], f32, tag="mba")
                        b = mbpool.tile([FPC, BPC], f32, tag="mbb")
                        d = mbpool.tile([FPC, BPC], f32, tag="mbd")
                        # seed from x so values are finite
                        nc.vector.tensor_scalar(
                            a[:], x, 1.0, None, Alu.mult
                        )  # ts 1-op imm
                        nc.vector.tensor_scalar(
                            b[:], x, sc, None, Alu.mult
                        )  # ts 1-op scalar-AP
                        nc.vector.tensor_scalar(
                            d[:], x, sc, 0.0, Alu.add, Alu.max
                        )  # ts 2-op
                        nc.vector.tensor_tensor(
                            d[:], a[:], b[:], Alu.mult
                        )  # TT mult
                        nc.vector.scalar_tensor_tensor(
                            d[:], a[:], sc, b[:], Alu.mult, Alu.mult
                        )  # stt
                        nc.gpsimd.tensor_tensor(
                            d[:], a[:], b[:], Alu.mult
                        )  # GPS TT
                        nc.scalar.activation(
                            d[:], a[:], Act.Square, bias=sc
                        )  # ACT
                        nc.vector.tensor_copy(d[:], a[:])  # copy

    nc.finalize()
    _prog_cache["nc"] = nc
    return nc


def _collapse_coeffs(knots: np.ndarray, weights: np.ndarray) -> np.ndarray:
    """c[f,k] = sum_{n+w=k} weights[f,n]*coef[f,n,w], computed in fp64."""
    kd = knots.astype(np.float64)
    wd = weights.astype(np.float64)
    idx = np.arange(NB)[:, None] + np.arange(W)[None, :]  # (NB, W)
    kwin = kd[:, idx]  # (F, NB, W)
    dif = kwin[..., None, :] - kwin[..., :, None] + np.eye(W)
    coef = 1.0 / np.prod(dif, axis=-1)  # (F, NB, W)
    c = np.zeros((kd.shape[0], NK), dtype=np.float64)
    for w in range(W):
        c[:, w : w + NB] += wd * coef[:, :, w]
    return c.astype(np.float32)


def kernel(x, knots, weights, _trace=False):
    x = np.asarray(x)
    knots = np.asarray(knots)
    weights = np.asarray(weights)
    c = _collapse_coeffs(knots, weights)
    negk = (-knots).astype(np.float32)

    nc = _build_program()
    in_maps = []
    for ci in range(NCORES):
        fq, bh = ci % FSH, ci // FSH
        fsl = slice(fq * FPC, (fq + 1) * FPC)
        bsl = slice(bh * BPC, (bh + 1) * BPC)
        xin = np.concatenate(
            [x[bsl, fsl].T.astype(np.float32), negk[fsl], c[fsl]], axis=1
        )
        in_maps.append({"xin": np.ascontiguousarray(xin)})

    res = run_bass_kernel_spmd(
        nc, in_maps, list(range(NCORES)), trace=_trace
    )
    out = np.empty((B, F), dtype=np.float32)
    for ci in range(NCORES):
        fq, bh = ci % FSH, ci // FSH
        fsl = slice(fq * FPC, (fq + 1) * FPC)
        bsl = slice(bh * BPC, (bh + 1) * BPC)
        out[bsl, fsl] = res.results[ci]["ot"].T
    if _trace:
        return out, res
    return out
